# revision 58
# baseline (speedup 1.0000x reference)
"""Trainium2 Bass kernel for GQA decode attention (nn_Attention_45844480917562).

Tensor-parallel over 8 NeuronCores: each core owns 4 query heads + 1 KV head
(wq/wk/wv column-sharded). Each core computes its PARTIAL output projection
(full [4096, 32] over its own 512 attention features); the host sums the 8
partials — no on-device collective (the 15us ncfw constant overhead per
collective dominated the old tail).

Key layout choices (driven by the TimelineSim cost model):
  - V cache stored fp8 e3m4 in natural [pos, hd] chunk order and used directly
    as the matmul STATIONARY operand: P@V emits [128 hd, 4 h] per chunk
    (4-cycle matmuls instead of 129-cycle), and V DMA bytes halve.
    K stays bf16 (e3m4 K measured too close to the 2e-2 gate).
  - Softmax denominator via ones-vector matmuls accumulated alongside P@V;
    the divide is a per-column broadcast built with a 1-partition ones matmul.
  - wo is streamed in 4 chunks interleaved into the KV stream so the single
    shared DMA pipe never stalls attention.
"""

import os
import sys
import math

sys.path.insert(0, "/opt/trn_rl_repo")

import numpy as np
import ml_dtypes

import concourse.bass as bass
import concourse.mybir as mybir
from concourse import tile, bacc, masks
from concourse.bass_utils import run_bass_kernel_spmd

# ---------------- problem constants ----------------
DIM = 4096
N_HEADS = 32
N_KV_HEADS = 8
HEAD_DIM = 128
NCORE = 8
HPC = N_HEADS // NCORE            # 4 query heads per core
QF = HPC * HEAD_DIM               # 512 features per core
BSZ = (16, 16)
SP = (2048, 1024)                 # start_pos per group
TOT_B = 32
NFULL = (SP[0] // 128, SP[1] // 128)   # full 128-pos chunks per group: 16, 8
KCH = DIM // 128                  # 32 contraction chunks
SPT = 4                           # samples per KV tile

DT = mybir.dt.bfloat16
FP8 = mybir.dt.float8e3
NPDT = ml_dtypes.bfloat16
NP8 = ml_dtypes.float8_e3m4
f32 = mybir.dt.float32

# each entry is one COMPUTE block; the inner tuple gives tapered DMA
# sub-chunk sizes (smaller arrivals at the stream tail, but one shared
# normalize pass per compute block)
BLOCKS = ([(4,), (4,), (4,), (4,)], [(4,), (4,), (4,), (2, 2)])
# wo is DMA'd in 4 one-head chunks; chunk i's dma_start is placed (in ACT
# program order) after sample-block WO_AFTER[i] so the transfer interleaves
# into the KV stream instead of starving it.  (g, block_idx) positions.
WO_AFTER = {(0, 0): (0, 1), (0, 1): (2, 3)}
# sample count after which each group's early wo-partial half is emitted —
# late enough that wo chunks are resident so the in-order PE queue never
# bubbles waiting on them
SPLIT = (12, 12)


def _build_nc():
    nc = bacc.Bacc(trn_type="TRN2", num_devices=NCORE, enable_asserts=True)

    # ---- I/O ----
    xh = nc.dram_tensor("xh", [128, KCH, TOT_B], DT, kind="ExternalInput")
    wqkv = nc.dram_tensor("wqkv", [128, KCH, QF + 2 * HEAD_DIM], DT, kind="ExternalInput")
    # wo in [local_c, f] layout: wo_cf[p, h, f] = wo[f, 512*r + h*128 + p]
    wo = nc.dram_tensor("wo", [128, HPC, DIM], DT, kind="ExternalInput")
    kt0 = nc.dram_tensor("kt0", [BSZ[0], 128, SP[0]], DT, kind="ExternalInput")
    kt1 = nc.dram_tensor("kt1", [BSZ[1], 128, SP[1]], DT, kind="ExternalInput")
    vp0 = nc.dram_tensor("vp0", [BSZ[0], 128, NFULL[0], 128], FP8, kind="ExternalInput")
    vp1 = nc.dram_tensor("vp1", [BSZ[1], 128, NFULL[1], 128], FP8, kind="ExternalInput")
    ropec = nc.dram_tensor("ropec", [128, TOT_B], f32, kind="ExternalInput")
    ropes = nc.dram_tensor("ropes", [128, TOT_B], f32, kind="ExternalInput")
    # flattened identity rows: selrows[0, 32*b + s] = (s == b), used to scatter
    # each sample's new-position probs to partition b via a 1-row matmul
    selrows = nc.dram_tensor("selrows", [1, TOT_B * TOT_B], DT, kind="ExternalInput")
    # y: per-core PARTIAL output projection, [g, fq, p, fi*16+b] with
    # f_global = 128*(8*fq+fi) + p.  Host un-permutes and sums over cores.
    y = nc.dram_tensor("y", [2, 4, 128, 128], f32, kind="ExternalOutput")

    WQKV_W = QF + 2 * HEAD_DIM  # 768
    SWAP_MASK = [i ^ 1 for i in range(32)]

    with tile.TileContext(nc) as tc:
        with tc.tile_pool(name="cpool", bufs=1) as cpool, \
             tc.tile_pool(name="wpool", bufs=2) as wpool, \
             tc.tile_pool(name="kvpool", bufs=4) as kvpool, \
             tc.tile_pool(name="apool", bufs=3) as apool, \
             tc.tile_pool(name="dpool", bufs=1, space="DRAM") as dpool:

            # ---------- constants ----------
            ident = cpool.tile([128, 128], f32)
            masks.make_identity(nc, ident[:])

            x_sb = cpool.tile([128, KCH * TOT_B], DT)
            nc.scalar.dma_start(x_sb[:].rearrange("p (c b) -> p c b", c=KCH), xh[:])
            ropec_sb = cpool.tile([128, TOT_B], f32)
            nc.scalar.dma_start(ropec_sb[:], ropec[:])
            ropes_sb = cpool.tile([128, TOT_B], f32)
            nc.scalar.dma_start(ropes_sb[:], ropes[:])

            ones128 = cpool.tile([128, 1], DT)
            nc.vector.memset(ones128[:], 1.0)
            ones1f = cpool.tile([1, 128], f32)
            nc.vector.memset(ones1f[:], 1.0)
            selrows_sb = cpool.tile([1, TOT_B * TOT_B], DT)
            nc.scalar.dma_start(selrows_sb[:], selrows[:])

            # ---------- phase A: QKV projection ----------
            with tc.tile_pool(name="ps_a", bufs=1, space="PSUM") as ps_a:
                qkv_ps = ps_a.tile([TOT_B, WQKV_W], f32)
                for P in range(4):
                    wq_t = wpool.tile([128, 8 * WQKV_W], DT, tag="wq", bufs=2)
                    nc.sync.dma_start(
                        wq_t[:].rearrange("p (c j) -> p c j", c=8),
                        wqkv[:, 8 * P:8 * P + 8, :],
                    )
                    for ci in range(8):
                        c = 8 * P + ci
                        lhs = x_sb[:, TOT_B * c:TOT_B * (c + 1)]
                        rhs = wq_t[:, WQKV_W * ci:WQKV_W * (ci + 1)]
                        nc.tensor.matmul(qkv_ps[:, 0:512], lhs, rhs[:, 0:512],
                                         start=(c == 0), stop=(c == KCH - 1))
                        nc.tensor.matmul(qkv_ps[:, 512:768], lhs, rhs[:, 512:768],
                                         start=(c == 0), stop=(c == KCH - 1))

                qkv_sb = cpool.tile([TOT_B, WQKV_W], f32)
                nc.scalar.copy(qkv_sb[:], qkv_ps[:])

            # wo weights, loaded in 4 one-head chunks interleaved into the KV
            # stream (see WO_AFTER)
            wo_all = wpool.tile([128, HPC * DIM], DT, tag="wo", bufs=1)

            psb_cm = tc.tile_pool(name="ps_b", bufs=2, space="PSUM")
            ps_b = psb_cm.__enter__()

            # new-position V rows (per sample), straight from the projection
            vnewdt = cpool.tile([TOT_B, HEAD_DIM], DT)
            nc.vector.tensor_copy(vnewdt[:], qkv_sb[:, 640:768])

            # ---------- transpose q heads + k, apply RoPE ----------
            qT4 = cpool.tile([128, HPC * TOT_B], DT)   # col = b*4 + h
            kTn = cpool.tile([128, TOT_B], DT)         # col = b
            for h in range(HPC + 1):                   # 4 q heads then k
                tp = ps_b.tile([128, TOT_B], f32, tag="tp")
                nc.tensor.transpose(tp[:], qkv_sb[:, 128 * h:128 * (h + 1)],
                                    ident[0:TOT_B, 0:TOT_B])
                t_sb = apool.tile([128, TOT_B], f32, tag="tr")
                nc.vector.tensor_copy(t_sb[:], tp[:])
                sw = apool.tile([128, TOT_B], f32, tag="sw")
                nc.vector.stream_shuffle(sw[:], t_sb[:], SWAP_MASK)
                t1 = apool.tile([128, TOT_B], f32, tag="t1")
                nc.vector.tensor_mul(t1[:], t_sb[:], ropec_sb[:])
                nc.vector.tensor_mul(sw[:], sw[:], ropes_sb[:])
                if h < HPC:
                    dest = qT4[:, h::HPC]
                else:
                    dest = kTn[:]
                nc.vector.tensor_add(dest, t1[:], sw[:])

            # ---------- phase B: attention over the KV cache ----------
            attnT = cpool.tile([128, HPC * TOT_B], DT)  # col = h*32 + b
            kts = (kt0, kt1)
            vps = (vp0, vp1)

            def emit_partials(g, pT_sb, b0, b1):
                # pt[f, b] = sum_c wo[f, c] * attn[b, c] for samples
                # [16g+b0, 16g+b1) — split in halves so the first half runs
                # under the KV stream and only the tail half gates the end
                w = b1 - b0
                for fq in range(4):
                    pt_ps = ps_b.tile([128, 128], f32, tag="tp")
                    for fi in range(8):
                        fb = 8 * fq + fi
                        for h in range(HPC):
                            nc.tensor.matmul(
                                pt_ps[:, w * fi:w * (fi + 1)],
                                wo_all[:, h * DIM + 128 * fb:h * DIM + 128 * (fb + 1)],
                                attnT[:, TOT_B * h + 16 * g + b0:TOT_B * h + 16 * g + b1],
                                start=(h == 0), stop=(h == HPC - 1))
                    dst = pT_sb[:, 128 * fq:128 * (fq + 1)].rearrange(
                        "p (fi b) -> p fi b", fi=8)[:, :, b0:b1]
                    src = pt_ps[:, 0:8 * w].rearrange("p (fi b) -> p fi b", fi=8)
                    if fq % 2 == 0:
                        nc.scalar.copy(dst, src)
                    else:
                        nc.vector.tensor_copy(dst, src)

            pT_tiles = [apool.tile([128, 4 * 128], f32, tag="pt", bufs=2,
                                   name=f"pT_sb{_g}")
                        for _g in range(2)]
            for g in range(2):
                npos = SP[g]
                nf = NFULL[g]
                ncol = 4 * nf
                vw = nf * 128
                pT_sb = pT_tiles[g]
                s_off = 0
                for bi, sub in enumerate(BLOCKS[g]):
                    blk = sum(sub)
                    ktile = kvpool.tile([128, SPT * SP[0]], DT, tag="kt")
                    vtile = kvpool.tile([128, SPT * NFULL[0] * 128], FP8, tag="vt")
                    off = 0
                    for sb_n in sub:
                        so = s_off + off
                        nc.sync.dma_start(
                            ktile[:, off * npos:(off + sb_n) * npos].rearrange(
                                "p (s n) -> p s n", s=sb_n),
                            kts[g][so:so + sb_n].rearrange("s p n -> p s n"),
                        )
                        nc.sync.dma_start(
                            vtile[:, off * vw:(off + sb_n) * vw].rearrange(
                                "p (s c d) -> p s c d", s=sb_n, c=nf),
                            vps[g][so:so + sb_n].rearrange("s p c d -> p s c d"),
                        )
                        off += sb_n
                    # one bank-sized psum tile per block: cols [0:16) P@V
                    # accums (4 per sample), [16:32) denominators (row 0),
                    # [32:48) sel scatters, [48:64) the reciprocal broadcast
                    ob = ps_b.tile([128, 16 * SPT], f32, tag="ob", bufs=4)
                    # all scores for the block first (no cross-engine waits on
                    # PE), then the exps pipeline on ACT while later samples'
                    # scores still run
                    sc_blk = ps_b.tile([128, 68 * SPT], f32, tag="sc")
                    prs = []
                    for j in range(blk):
                        b = 16 * g + s_off + j
                        ks = ktile[:, j * npos:(j + 1) * npos]
                        q_b = qT4[:, HPC * b:HPC * (b + 1)]
                        sc_ps = sc_blk[:, 68 * j:68 * (j + 1)]
                        for c in range(nf):
                            nc.tensor.matmul(sc_ps[:, 4 * c:4 * c + 4],
                                             ks[:, 128 * c:128 * (c + 1)], q_b,
                                             start=True, stop=True)
                        nc.tensor.matmul(sc_ps[0:1, ncol:ncol + 4],
                                         kTn[:, b:b + 1], q_b,
                                         start=True, stop=True)
                    for j in range(blk):
                        sc_ps = sc_blk[:, 68 * j:68 * (j + 1)]
                        pr = apool.tile([128, 68], DT, tag="pr", bufs=6)
                        nc.scalar.activation(pr[:, 0:ncol + 4], sc_ps[:, 0:ncol + 4],
                                             mybir.ActivationFunctionType.Exp)
                        prs.append(pr)
                    for j in range(blk):
                        b = 16 * g + s_off + j
                        vs = vtile[:, j * vw:(j + 1) * vw]
                        pr = prs[j]

                        # new-position probs scattered to partition b (via a
                        # 1-row identity matmul; engines can't address a lone
                        # partition b directly), so the new-pos P@V / denom
                        # terms batch over vnewdt rows
                        sel_ps = ob[0:TOT_B, 8 * SPT + 4 * j:8 * SPT + 4 * j + 4]
                        nc.tensor.matmul(sel_ps,
                                         selrows_sb[0:1, TOT_B * b:TOT_B * (b + 1)],
                                         pr[0:1, ncol:ncol + 4],
                                         start=True, stop=True)
                        sel = apool.tile([TOT_B, 4], DT, tag="sel", bufs=6)
                        nc.vector.tensor_copy(sel[:], sel_ps)

                        # denominators BEFORE P@V so the reciprocal (DVE)
                        # overlaps the P@V matmuls on the in-order PE queue
                        dslice = ob[0:1, 4 * SPT + 4 * j:4 * SPT + 4 * j + 4]
                        for c in range(nf):
                            nc.tensor.matmul(dslice, ones128[:], pr[:, 4 * c:4 * c + 4],
                                             start=(c == 0), stop=False)
                        nc.tensor.matmul(dslice, ones128[0:TOT_B, :], sel[:],
                                         start=False, stop=True)

                        o_ps = ob[:, 4 * j:4 * j + 4]
                        for c in range(nf):
                            nc.tensor.matmul(o_ps, vs[:, 128 * c:128 * (c + 1)],
                                             pr[:, 4 * c:4 * c + 4],
                                             start=(c == 0), stop=False)
                        nc.tensor.matmul(o_ps, vnewdt[:], sel[:],
                                         start=False, stop=True)

                    # batched reciprocal + per-column broadcast for the block
                    rec = apool.tile([1, 4 * SPT], f32, tag="rec")
                    nc.vector.reciprocal(rec[0:1, 0:4 * blk],
                                         ob[0:1, 4 * SPT:4 * SPT + 4 * blk])
                    rb_sb = apool.tile([128, 4 * SPT], f32, tag="rbs")
                    nc.gpsimd.partition_broadcast(rb_sb[:, 0:4 * blk],
                                                  rec[0:1, 0:4 * blk])
                    for j in range(blk):
                        b = 16 * g + s_off + j
                        nc.vector.tensor_mul(attnT[:, b::TOT_B],
                                             ob[:, 4 * j:4 * j + 4],
                                             rb_sb[:, 4 * j:4 * j + 4])
                    s_off += blk

                    for wi in WO_AFTER.get((g, bi), ()):
                        nc.scalar.dma_start(wo_all[:, DIM * wi:DIM * (wi + 1)],
                                            wo[:, wi, :])
                    if s_off == SPLIT[g]:
                        emit_partials(g, pT_sb, 0, SPLIT[g])
                    if g == 1 and bi == 2:
                        # g0's writeback, mid-g1-stream: its data has long been
                        # ready, and placing it here keeps it out of the final
                        # basic block whose entry barrier would delay it
                        nc.sync.dma_start(
                            y[0].rearrange("f p c -> p f c"),
                            pT_tiles[0][:].rearrange("p (f c) -> p f c", f=4),
                        )

                emit_partials(g, pT_sb, SPLIT[g], 16)

            # y1 in two halves on separate rings: each half's descgen+DGE
            # overlaps the other half's partial matmuls/copies
            nc.sync.dma_start(
                y[1, 0:2].rearrange("f p c -> p f c"),
                pT_tiles[1][:, 0:256].rearrange("p (f c) -> p f c", f=2),
            )
            nc.scalar.dma_start(
                y[1, 2:4].rearrange("f p c -> p f c"),
                pT_tiles[1][:, 256:512].rearrange("p (f c) -> p f c", f=2),
            )
            psb_cm.__exit__(None, None, None)

    nc.finalize()
    return nc


_NC_CACHE = None


def _get_nc():
    global _NC_CACHE
    if _NC_CACHE is None:
        _NC_CACHE = _build_nc()
    return _NC_CACHE


def _prep_inputs(inputs):
    """Shard + lay out the full inputs for the 8 cores."""
    x = np.asarray(inputs["x"], np.float32)
    wq = np.asarray(inputs["wq"], np.float32)
    wk = np.asarray(inputs["wk"], np.float32)
    wv = np.asarray(inputs["wv"], np.float32)
    wo = np.asarray(inputs["wo"], np.float32)
    fc = np.asarray(inputs["freqs_cos"], np.float32)
    fs = np.asarray(inputs["freqs_sin"], np.float32)
    caches = (
        (np.asarray(inputs["cache_k0"], np.float32), np.asarray(inputs["cache_v0"], np.float32)),
        (np.asarray(inputs["cache_k1"], np.float32), np.asarray(inputs["cache_v1"], np.float32)),
    )

    x_flat = x.reshape(TOT_B, DIM)
    xh = np.ascontiguousarray(
        x_flat.T.reshape(KCH, 128, TOT_B).transpose(1, 0, 2)
    ).astype(NPDT)

    # RoPE tables: per-column position (2048 for tokens 0-15, 1024 for 16-31)
    C = np.empty((128, TOT_B), np.float32)
    S = np.empty((128, TOT_B), np.float32)
    for g in range(2):
        cos = fc[SP[g]]
        sin = fs[SP[g]]
        cols = slice(16 * g, 16 * (g + 1))
        C[0::2, cols] = cos[:, None]
        C[1::2, cols] = cos[:, None]
        S[0::2, cols] = -sin[:, None]
        S[1::2, cols] = sin[:, None]

    scale = 1.0 / math.sqrt(HEAD_DIM)
    selrows = np.eye(TOT_B, dtype=NPDT).reshape(1, TOT_B * TOT_B)

    def _prep_core(r):
        w_q = wq[QF * r:QF * (r + 1)] * scale
        w_k = wk[HEAD_DIM * r:HEAD_DIM * (r + 1)]
        w_v = wv[HEAD_DIM * r:HEAD_DIM * (r + 1)]
        wqkvT = np.concatenate([w_q, w_k, w_v], axis=0).T  # [4096, 768]
        wqkv_hp = np.ascontiguousarray(
            wqkvT.reshape(KCH, 128, 768).transpose(1, 0, 2)
        ).astype(NPDT)

        # wo_cf[local_c, f] = wo[f, 512r + local_c]  -> [128, HPC, 4096]
        wo_cf = wo[:, QF * r:QF * (r + 1)].T  # [512, 4096]
        wo_hp = np.ascontiguousarray(
            wo_cf.reshape(HPC, 128, DIM).transpose(1, 0, 2)
        ).astype(NPDT)

        m = {"xh": xh, "wqkv": wqkv_hp, "wo": wo_hp,
             "ropec": C, "ropes": S, "selrows": selrows}
        for g in range(2):
            ck, cv = caches[g]
            npos = SP[g]
            nf = NFULL[g]
            kslab = ck[:, :npos, r, :].astype(NPDT)       # [16, npos, 128]
            kt = np.ascontiguousarray(kslab.transpose(0, 2, 1))  # [16, 128, npos]
            vslab = cv[:, :npos, r, :].astype(NP8).reshape(BSZ[g], nf, 128, HEAD_DIM)
            vp = np.ascontiguousarray(vslab.transpose(0, 2, 1, 3))  # [16, 128, nf, 128]
            m[f"kt{g}"] = kt
            m[f"vp{g}"] = vp
        return m

    from concurrent.futures import ThreadPoolExecutor
    with ThreadPoolExecutor(max_workers=NCORE) as ex:
        in_maps = list(ex.map(_prep_core, range(NCORE)))
    return in_maps


def _run(inputs, trace=False):
    nc = _get_nc()
    in_maps = _prep_inputs(inputs)
    res = run_bass_kernel_spmd(nc, in_maps, core_ids=list(range(NCORE)), trace=trace)
    # each core returns its PARTIAL projection y[g, fq, p, fi*16+b] with
    # f = 128*(8*fq+fi) + p; un-permute and sum over cores.
    total = None
    for r in range(NCORE):
        yr = res.results[r]["y"].reshape(2, 4, 128, 8, 16)
        part = yr.transpose(1, 3, 2, 0, 4).reshape(DIM, TOT_B)
        total = part if total is None else total + part
    out = np.ascontiguousarray(total.T).reshape(TOT_B, 1, DIM).astype(np.float32)
    return out, res


def kernel(**inputs):
    try:
        out, _ = _run(inputs, trace=False)
    except Exception:
        # transient NRT/axon hiccups have been observed to recover on retry
        out, _ = _run(inputs, trace=False)
    return out


# revision 59
# speedup vs baseline: 1.0083x; 1.0083x over previous
"""Trainium2 Bass kernel for GQA decode attention (nn_Attention_45844480917562).

Tensor-parallel over 8 NeuronCores: each core owns 4 query heads + 1 KV head
(wq/wk/wv column-sharded). Each core computes its PARTIAL output projection
(full [4096, 32] over its own 512 attention features); the host sums the 8
partials — no on-device collective (the 15us ncfw constant overhead per
collective dominated the old tail).

Key layout choices (driven by the TimelineSim cost model):
  - V cache stored fp8 e3m4 in natural [pos, hd] chunk order and used directly
    as the matmul STATIONARY operand: P@V emits [128 hd, 4 h] per chunk
    (4-cycle matmuls instead of 129-cycle), and V DMA bytes halve.
    K stays bf16 (e3m4 K measured too close to the 2e-2 gate).
  - Softmax denominator via ones-vector matmuls accumulated alongside P@V;
    the divide is a per-column broadcast built with a 1-partition ones matmul.
  - wo is streamed in 4 chunks interleaved into the KV stream so the single
    shared DMA pipe never stalls attention.
"""

import os
import sys
import math

sys.path.insert(0, "/opt/trn_rl_repo")

import numpy as np
import ml_dtypes

import concourse.bass as bass
import concourse.mybir as mybir
from concourse import tile, bacc, masks
from concourse.bass_utils import run_bass_kernel_spmd

# ---------------- problem constants ----------------
DIM = 4096
N_HEADS = 32
N_KV_HEADS = 8
HEAD_DIM = 128
NCORE = 8
HPC = N_HEADS // NCORE            # 4 query heads per core
QF = HPC * HEAD_DIM               # 512 features per core
BSZ = (16, 16)
SP = (2048, 1024)                 # start_pos per group
TOT_B = 32
NFULL = (SP[0] // 128, SP[1] // 128)   # full 128-pos chunks per group: 16, 8
KCH = DIM // 128                  # 32 contraction chunks
SPT = 4                           # samples per KV tile

DT = mybir.dt.bfloat16
FP8 = mybir.dt.float8e3
NPDT = ml_dtypes.bfloat16
NP8 = ml_dtypes.float8_e3m4
f32 = mybir.dt.float32

# each entry is one COMPUTE block; the inner tuple gives tapered DMA
# sub-chunk sizes (smaller arrivals at the stream tail, but one shared
# normalize pass per compute block)
BLOCKS = ([(4,), (4,), (4,), (4,)], [(4,), (4,), (4,), (2,), (2,)])
# wo is DMA'd in 4 one-head chunks; chunk i's dma_start is placed (in ACT
# program order) after sample-block WO_AFTER[i] so the transfer interleaves
# into the KV stream instead of starving it.  (g, block_idx) positions.
WO_AFTER = {(0, 0): (0, 1), (0, 1): (2, 3)}
# sample count after which each group's early wo-partial half is emitted —
# late enough that wo chunks are resident so the in-order PE queue never
# bubbles waiting on them
SPLIT = (12, 12)


def _build_nc():
    nc = bacc.Bacc(trn_type="TRN2", num_devices=NCORE, enable_asserts=True)

    # ---- I/O ----
    xh = nc.dram_tensor("xh", [128, KCH, TOT_B], DT, kind="ExternalInput")
    wqkv = nc.dram_tensor("wqkv", [128, KCH, QF + 2 * HEAD_DIM], DT, kind="ExternalInput")
    # wo in [local_c, f] layout: wo_cf[p, h, f] = wo[f, 512*r + h*128 + p]
    wo = nc.dram_tensor("wo", [128, HPC, DIM], DT, kind="ExternalInput")
    kt0 = nc.dram_tensor("kt0", [BSZ[0], 128, SP[0]], DT, kind="ExternalInput")
    kt1 = nc.dram_tensor("kt1", [BSZ[1], 128, SP[1]], DT, kind="ExternalInput")
    vp0 = nc.dram_tensor("vp0", [BSZ[0], 128, NFULL[0], 128], FP8, kind="ExternalInput")
    vp1 = nc.dram_tensor("vp1", [BSZ[1], 128, NFULL[1], 128], FP8, kind="ExternalInput")
    ropec = nc.dram_tensor("ropec", [128, TOT_B], f32, kind="ExternalInput")
    ropes = nc.dram_tensor("ropes", [128, TOT_B], f32, kind="ExternalInput")
    # flattened identity rows: selrows[0, 32*b + s] = (s == b), used to scatter
    # each sample's new-position probs to partition b via a 1-row matmul
    selrows = nc.dram_tensor("selrows", [1, TOT_B * TOT_B], DT, kind="ExternalInput")
    # y: per-core PARTIAL output projection, [g, fq, p, fi*16+b] with
    # f_global = 128*(8*fq+fi) + p.  Host un-permutes and sums over cores.
    y = nc.dram_tensor("y", [2, 4, 128, 128], f32, kind="ExternalOutput")

    WQKV_W = QF + 2 * HEAD_DIM  # 768
    SWAP_MASK = [i ^ 1 for i in range(32)]

    with tile.TileContext(nc) as tc:
        with tc.tile_pool(name="cpool", bufs=1) as cpool, \
             tc.tile_pool(name="wpool", bufs=2) as wpool, \
             tc.tile_pool(name="kvpool", bufs=4) as kvpool, \
             tc.tile_pool(name="apool", bufs=3) as apool, \
             tc.tile_pool(name="dpool", bufs=1, space="DRAM") as dpool:

            # ---------- constants ----------
            ident = cpool.tile([128, 128], f32)
            masks.make_identity(nc, ident[:])

            x_sb = cpool.tile([128, KCH * TOT_B], DT)
            nc.scalar.dma_start(x_sb[:].rearrange("p (c b) -> p c b", c=KCH), xh[:])
            ropec_sb = cpool.tile([128, TOT_B], f32)
            nc.scalar.dma_start(ropec_sb[:], ropec[:])
            ropes_sb = cpool.tile([128, TOT_B], f32)
            nc.scalar.dma_start(ropes_sb[:], ropes[:])

            ones128 = cpool.tile([128, 1], DT)
            nc.vector.memset(ones128[:], 1.0)
            ones1f = cpool.tile([1, 128], f32)
            nc.vector.memset(ones1f[:], 1.0)
            selrows_sb = cpool.tile([1, TOT_B * TOT_B], DT)
            nc.scalar.dma_start(selrows_sb[:], selrows[:])

            # ---------- phase A: QKV projection ----------
            with tc.tile_pool(name="ps_a", bufs=1, space="PSUM") as ps_a:
                qkv_ps = ps_a.tile([TOT_B, WQKV_W], f32)
                for P in range(4):
                    wq_t = wpool.tile([128, 8 * WQKV_W], DT, tag="wq", bufs=2)
                    nc.sync.dma_start(
                        wq_t[:].rearrange("p (c j) -> p c j", c=8),
                        wqkv[:, 8 * P:8 * P + 8, :],
                    )
                    for ci in range(8):
                        c = 8 * P + ci
                        lhs = x_sb[:, TOT_B * c:TOT_B * (c + 1)]
                        rhs = wq_t[:, WQKV_W * ci:WQKV_W * (ci + 1)]
                        nc.tensor.matmul(qkv_ps[:, 0:512], lhs, rhs[:, 0:512],
                                         start=(c == 0), stop=(c == KCH - 1))
                        nc.tensor.matmul(qkv_ps[:, 512:768], lhs, rhs[:, 512:768],
                                         start=(c == 0), stop=(c == KCH - 1))

                qkv_sb = cpool.tile([TOT_B, WQKV_W], f32)
                nc.scalar.copy(qkv_sb[:], qkv_ps[:])

            # wo weights, loaded in 4 one-head chunks interleaved into the KV
            # stream (see WO_AFTER)
            wo_all = wpool.tile([128, HPC * DIM], DT, tag="wo", bufs=1)

            psb_cm = tc.tile_pool(name="ps_b", bufs=2, space="PSUM")
            ps_b = psb_cm.__enter__()

            # new-position V rows (per sample), straight from the projection
            vnewdt = cpool.tile([TOT_B, HEAD_DIM], DT)
            nc.vector.tensor_copy(vnewdt[:], qkv_sb[:, 640:768])

            # ---------- transpose q heads + k, apply RoPE ----------
            qT4 = cpool.tile([128, HPC * TOT_B], DT)   # col = b*4 + h
            kTn = cpool.tile([128, TOT_B], DT)         # col = b
            for h in range(HPC + 1):                   # 4 q heads then k
                tp = ps_b.tile([128, TOT_B], f32, tag="tp")
                nc.tensor.transpose(tp[:], qkv_sb[:, 128 * h:128 * (h + 1)],
                                    ident[0:TOT_B, 0:TOT_B])
                t_sb = apool.tile([128, TOT_B], f32, tag="tr")
                nc.vector.tensor_copy(t_sb[:], tp[:])
                sw = apool.tile([128, TOT_B], f32, tag="sw")
                nc.vector.stream_shuffle(sw[:], t_sb[:], SWAP_MASK)
                t1 = apool.tile([128, TOT_B], f32, tag="t1")
                nc.vector.tensor_mul(t1[:], t_sb[:], ropec_sb[:])
                nc.vector.tensor_mul(sw[:], sw[:], ropes_sb[:])
                if h < HPC:
                    dest = qT4[:, h::HPC]
                else:
                    dest = kTn[:]
                nc.vector.tensor_add(dest, t1[:], sw[:])

            # ---------- phase B: attention over the KV cache ----------
            attnT = cpool.tile([128, HPC * TOT_B], DT)  # col = h*32 + b
            kts = (kt0, kt1)
            vps = (vp0, vp1)

            def emit_partials(g, pT_sb, b0, b1):
                # pt[f, b] = sum_c wo[f, c] * attn[b, c] for samples
                # [16g+b0, 16g+b1) — split in halves so the first half runs
                # under the KV stream and only the tail half gates the end
                w = b1 - b0
                for fq in range(4):
                    pt_ps = ps_b.tile([128, 128], f32, tag="tp")
                    for fi in range(8):
                        fb = 8 * fq + fi
                        for h in range(HPC):
                            nc.tensor.matmul(
                                pt_ps[:, w * fi:w * (fi + 1)],
                                wo_all[:, h * DIM + 128 * fb:h * DIM + 128 * (fb + 1)],
                                attnT[:, TOT_B * h + 16 * g + b0:TOT_B * h + 16 * g + b1],
                                start=(h == 0), stop=(h == HPC - 1))
                    dst = pT_sb[:, 128 * fq:128 * (fq + 1)].rearrange(
                        "p (fi b) -> p fi b", fi=8)[:, :, b0:b1]
                    src = pt_ps[:, 0:8 * w].rearrange("p (fi b) -> p fi b", fi=8)
                    if fq % 2 == 0:
                        nc.scalar.copy(dst, src)
                    else:
                        nc.vector.tensor_copy(dst, src)

            pT_tiles = [apool.tile([128, 4 * 128], f32, tag="pt", bufs=2,
                                   name=f"pT_sb{_g}")
                        for _g in range(2)]
            for g in range(2):
                npos = SP[g]
                nf = NFULL[g]
                ncol = 4 * nf
                vw = nf * 128
                pT_sb = pT_tiles[g]
                s_off = 0
                for bi, sub in enumerate(BLOCKS[g]):
                    blk = sum(sub)
                    ktile = kvpool.tile([128, SPT * SP[0]], DT, tag="kt")
                    vtile = kvpool.tile([128, SPT * NFULL[0] * 128], FP8, tag="vt")
                    off = 0
                    for sb_n in sub:
                        so = s_off + off
                        nc.sync.dma_start(
                            ktile[:, off * npos:(off + sb_n) * npos].rearrange(
                                "p (s n) -> p s n", s=sb_n),
                            kts[g][so:so + sb_n].rearrange("s p n -> p s n"),
                        )
                        nc.sync.dma_start(
                            vtile[:, off * vw:(off + sb_n) * vw].rearrange(
                                "p (s c d) -> p s c d", s=sb_n, c=nf),
                            vps[g][so:so + sb_n].rearrange("s p c d -> p s c d"),
                        )
                        off += sb_n
                    # one bank-sized psum tile per block: cols [0:16) P@V
                    # accums (4 per sample), [16:32) denominators (row 0),
                    # [32:48) sel scatters, [48:64) the reciprocal broadcast
                    ob = ps_b.tile([128, 16 * SPT], f32, tag="ob", bufs=4)
                    # all scores for the block first (no cross-engine waits on
                    # PE), then the exps pipeline on ACT while later samples'
                    # scores still run
                    sc_blk = ps_b.tile([128, 68 * SPT], f32, tag="sc")
                    prs = []
                    for j in range(blk):
                        b = 16 * g + s_off + j
                        ks = ktile[:, j * npos:(j + 1) * npos]
                        q_b = qT4[:, HPC * b:HPC * (b + 1)]
                        sc_ps = sc_blk[:, 68 * j:68 * (j + 1)]
                        for c in range(nf):
                            nc.tensor.matmul(sc_ps[:, 4 * c:4 * c + 4],
                                             ks[:, 128 * c:128 * (c + 1)], q_b,
                                             start=True, stop=True)
                        nc.tensor.matmul(sc_ps[0:1, ncol:ncol + 4],
                                         kTn[:, b:b + 1], q_b,
                                         start=True, stop=True)
                    for j in range(blk):
                        sc_ps = sc_blk[:, 68 * j:68 * (j + 1)]
                        pr = apool.tile([128, 68], DT, tag="pr", bufs=6)
                        nc.scalar.activation(pr[:, 0:ncol + 4], sc_ps[:, 0:ncol + 4],
                                             mybir.ActivationFunctionType.Exp)
                        prs.append(pr)
                    for j in range(blk):
                        b = 16 * g + s_off + j
                        vs = vtile[:, j * vw:(j + 1) * vw]
                        pr = prs[j]

                        # new-position probs scattered to partition b (via a
                        # 1-row identity matmul; engines can't address a lone
                        # partition b directly), so the new-pos P@V / denom
                        # terms batch over vnewdt rows
                        sel_ps = ob[0:TOT_B, 8 * SPT + 4 * j:8 * SPT + 4 * j + 4]
                        nc.tensor.matmul(sel_ps,
                                         selrows_sb[0:1, TOT_B * b:TOT_B * (b + 1)],
                                         pr[0:1, ncol:ncol + 4],
                                         start=True, stop=True)
                        sel = apool.tile([TOT_B, 4], DT, tag="sel", bufs=6)
                        nc.vector.tensor_copy(sel[:], sel_ps)

                        # denominators BEFORE P@V so the reciprocal (DVE)
                        # overlaps the P@V matmuls on the in-order PE queue
                        dslice = ob[0:1, 4 * SPT + 4 * j:4 * SPT + 4 * j + 4]
                        for c in range(nf):
                            nc.tensor.matmul(dslice, ones128[:], pr[:, 4 * c:4 * c + 4],
                                             start=(c == 0), stop=False)
                        nc.tensor.matmul(dslice, ones128[0:TOT_B, :], sel[:],
                                         start=False, stop=True)

                        o_ps = ob[:, 4 * j:4 * j + 4]
                        for c in range(nf):
                            nc.tensor.matmul(o_ps, vs[:, 128 * c:128 * (c + 1)],
                                             pr[:, 4 * c:4 * c + 4],
                                             start=(c == 0), stop=False)
                        nc.tensor.matmul(o_ps, vnewdt[:], sel[:],
                                         start=False, stop=True)

                    # batched reciprocal + per-column broadcast for the block
                    rec = apool.tile([1, 4 * SPT], f32, tag="rec")
                    nc.vector.reciprocal(rec[0:1, 0:4 * blk],
                                         ob[0:1, 4 * SPT:4 * SPT + 4 * blk])
                    rb_sb = apool.tile([128, 4 * SPT], f32, tag="rbs")
                    nc.gpsimd.partition_broadcast(rb_sb[:, 0:4 * blk],
                                                  rec[0:1, 0:4 * blk])
                    for j in range(blk):
                        b = 16 * g + s_off + j
                        nc.vector.tensor_mul(attnT[:, b::TOT_B],
                                             ob[:, 4 * j:4 * j + 4],
                                             rb_sb[:, 4 * j:4 * j + 4])
                    s_off += blk

                    for wi in WO_AFTER.get((g, bi), ()):
                        nc.scalar.dma_start(wo_all[:, DIM * wi:DIM * (wi + 1)],
                                            wo[:, wi, :])
                    if s_off == SPLIT[g]:
                        emit_partials(g, pT_sb, 0, SPLIT[g])
                    if g == 1 and bi == 2:
                        # g0's writeback, mid-g1-stream: its data has long been
                        # ready, and placing it here keeps it out of the final
                        # basic block whose entry barrier would delay it
                        nc.sync.dma_start(
                            y[0].rearrange("f p c -> p f c"),
                            pT_tiles[0][:].rearrange("p (f c) -> p f c", f=4),
                        )

                emit_partials(g, pT_sb, SPLIT[g], 16)

            # y1 in two halves on separate rings: each half's descgen+DGE
            # overlaps the other half's partial matmuls/copies
            nc.sync.dma_start(
                y[1, 0:2].rearrange("f p c -> p f c"),
                pT_tiles[1][:, 0:256].rearrange("p (f c) -> p f c", f=2),
            )
            nc.scalar.dma_start(
                y[1, 2:4].rearrange("f p c -> p f c"),
                pT_tiles[1][:, 256:512].rearrange("p (f c) -> p f c", f=2),
            )
            psb_cm.__exit__(None, None, None)

    nc.finalize()
    return nc


_NC_CACHE = None


def _get_nc():
    global _NC_CACHE
    if _NC_CACHE is None:
        _NC_CACHE = _build_nc()
    return _NC_CACHE


def _prep_inputs(inputs):
    """Shard + lay out the full inputs for the 8 cores."""
    x = np.asarray(inputs["x"], np.float32)
    wq = np.asarray(inputs["wq"], np.float32)
    wk = np.asarray(inputs["wk"], np.float32)
    wv = np.asarray(inputs["wv"], np.float32)
    wo = np.asarray(inputs["wo"], np.float32)
    fc = np.asarray(inputs["freqs_cos"], np.float32)
    fs = np.asarray(inputs["freqs_sin"], np.float32)
    caches = (
        (np.asarray(inputs["cache_k0"], np.float32), np.asarray(inputs["cache_v0"], np.float32)),
        (np.asarray(inputs["cache_k1"], np.float32), np.asarray(inputs["cache_v1"], np.float32)),
    )

    x_flat = x.reshape(TOT_B, DIM)
    xh = np.ascontiguousarray(
        x_flat.T.reshape(KCH, 128, TOT_B).transpose(1, 0, 2)
    ).astype(NPDT)

    # RoPE tables: per-column position (2048 for tokens 0-15, 1024 for 16-31)
    C = np.empty((128, TOT_B), np.float32)
    S = np.empty((128, TOT_B), np.float32)
    for g in range(2):
        cos = fc[SP[g]]
        sin = fs[SP[g]]
        cols = slice(16 * g, 16 * (g + 1))
        C[0::2, cols] = cos[:, None]
        C[1::2, cols] = cos[:, None]
        S[0::2, cols] = -sin[:, None]
        S[1::2, cols] = sin[:, None]

    scale = 1.0 / math.sqrt(HEAD_DIM)
    selrows = np.eye(TOT_B, dtype=NPDT).reshape(1, TOT_B * TOT_B)

    def _prep_core(r):
        w_q = wq[QF * r:QF * (r + 1)] * scale
        w_k = wk[HEAD_DIM * r:HEAD_DIM * (r + 1)]
        w_v = wv[HEAD_DIM * r:HEAD_DIM * (r + 1)]
        wqkvT = np.concatenate([w_q, w_k, w_v], axis=0).T  # [4096, 768]
        wqkv_hp = np.ascontiguousarray(
            wqkvT.reshape(KCH, 128, 768).transpose(1, 0, 2)
        ).astype(NPDT)

        # wo_cf[local_c, f] = wo[f, 512r + local_c]  -> [128, HPC, 4096]
        wo_cf = wo[:, QF * r:QF * (r + 1)].T  # [512, 4096]
        wo_hp = np.ascontiguousarray(
            wo_cf.reshape(HPC, 128, DIM).transpose(1, 0, 2)
        ).astype(NPDT)

        m = {"xh": xh, "wqkv": wqkv_hp, "wo": wo_hp,
             "ropec": C, "ropes": S, "selrows": selrows}
        for g in range(2):
            ck, cv = caches[g]
            npos = SP[g]
            nf = NFULL[g]
            kslab = ck[:, :npos, r, :].astype(NPDT)       # [16, npos, 128]
            kt = np.ascontiguousarray(kslab.transpose(0, 2, 1))  # [16, 128, npos]
            vslab = cv[:, :npos, r, :].astype(NP8).reshape(BSZ[g], nf, 128, HEAD_DIM)
            vp = np.ascontiguousarray(vslab.transpose(0, 2, 1, 3))  # [16, 128, nf, 128]
            m[f"kt{g}"] = kt
            m[f"vp{g}"] = vp
        return m

    from concurrent.futures import ThreadPoolExecutor
    with ThreadPoolExecutor(max_workers=NCORE) as ex:
        in_maps = list(ex.map(_prep_core, range(NCORE)))
    return in_maps


def _run(inputs, trace=False):
    nc = _get_nc()
    in_maps = _prep_inputs(inputs)
    res = run_bass_kernel_spmd(nc, in_maps, core_ids=list(range(NCORE)), trace=trace)
    # each core returns its PARTIAL projection y[g, fq, p, fi*16+b] with
    # f = 128*(8*fq+fi) + p; un-permute and sum over cores.
    total = None
    for r in range(NCORE):
        yr = res.results[r]["y"].reshape(2, 4, 128, 8, 16)
        part = yr.transpose(1, 3, 2, 0, 4).reshape(DIM, TOT_B)
        total = part if total is None else total + part
    out = np.ascontiguousarray(total.T).reshape(TOT_B, 1, DIM).astype(np.float32)
    return out, res


def kernel(**inputs):
    try:
        out, _ = _run(inputs, trace=False)
    except Exception:
        # transient NRT/axon hiccups have been observed to recover on retry
        out, _ = _run(inputs, trace=False)
    return out
